# revision 5
# baseline (speedup 1.0000x reference)
"""AWQ linear (int4 group-quantized) matmul on 8 Trainium2 NeuronCores.

out[m, n] = sum_k x[m, k] * W[n, k] + bias[n]
W[n, k] = (q4[n, k] - qzeros[n, k//128]) * qscales[n, k//128]

Column-parallel: shard N=11008 across 8 cores (1376 each), replicate x.
Per core:
  - host repacks qweight nibbles to k-major uint8 [K, Ns] and swizzles
    x^T (bf16) so every (k-group, m-tile) slab is one contiguous DMA
  - device dequantizes W^T[k, n] = (q4 - z)*s into resident SBUF bf16:
    GPSIMD broadcasts the per-group scale/zero rows across partitions,
    DVE does the two-tensor affine (exact fp32 math, bf16 store)
  - matmuls accumulate over k in 4 splits of 8 k-tiles so the PE can
    start while later groups still dequantize; partial sums accumulate
    in SBUF via gap-filler (ACT/DVE) tensor adds, bias fused in split 0
"""

import os

import numpy as np
import ml_dtypes

M, K, NFULL = 4096, 4096, 11008
NCORES = 8
NS = NFULL // NCORES          # 1376 out-features per core
P = 128                       # partitions; also the quant group size
MM_FREE = 512                 # psum bank limit (fp32)
KS_GROUP = 8                  # k-tiles per accumulation split
MT_BLOCK = 4                  # m-tiles per outsb residency block

LAST_RESULTS = None           # BassKernelResults of the last kernel() call


def build_nc(k=K, m=M, ns=NS, n_cores=NCORES, ks_group=KS_GROUP, mt_block=MT_BLOCK):
    """Build + compile the per-core Bass program (SPMD: same NEFF on all cores)."""
    import concourse.mybir as mybir
    import concourse.tile as tile
    from concourse import bacc

    kt_n = k // P
    mt_n = m // P
    assert kt_n % ks_group == 0 and mt_n % mt_block == 0
    ks_n = kt_n // ks_group
    blk_n = mt_n // mt_block
    chunks = [(i, min(MM_FREE, ns - i)) for i in range(0, ns, MM_FREE)]

    f32 = mybir.dt.float32
    bf16 = mybir.dt.bfloat16
    u8 = mybir.dt.uint8
    ADD = mybir.AluOpType.add
    SUB = mybir.AluOpType.subtract
    MUL = mybir.AluOpType.mult

    nc = bacc.Bacc("TRN2", num_devices=n_cores)
    # xt rows are (ks, mt, p) so each (ks, mt) slab is contiguous [128, ks_group*128]
    xt = nc.dram_tensor("xt", [ks_n * mt_n * P, ks_group * P], bf16, kind="ExternalInput")
    q4 = nc.dram_tensor("q4", [k, ns], u8, kind="ExternalInput")
    scl = nc.dram_tensor("scl", [kt_n, ns], f32, kind="ExternalInput")
    zro = nc.dram_tensor("zro", [kt_n, ns], f32, kind="ExternalInput")
    bias = nc.dram_tensor("bias", [1, ns], f32, kind="ExternalInput")
    out = nc.dram_tensor("out", [m, ns], f32, kind="ExternalOutput")

    with tile.TileContext(nc) as tc:
        with (
            tc.tile_pool(name="persist", bufs=1) as persist,
            tc.tile_pool(name="dq", bufs=2) as dq,
            tc.tile_pool(name="xp", bufs=3) as xp,
            tc.tile_pool(name="op", bufs=mt_block + 2) as op,
            tc.tile_pool(name="ps", bufs=6, space="PSUM") as ps,
        ):
            w_all = persist.tile([P, kt_n, ns], bf16)
            bias_row = persist.tile([1, ns], f32)
            bias_exp = persist.tile([P, ns], f32)
            nc.sync.dma_start(bias_row[:], bias.ap()[:, :])
            nc.gpsimd.partition_broadcast(bias_exp[:], bias_row[:])

            def dequant_group(ks):
                for kt in range(ks * ks_group, (ks + 1) * ks_group):
                    q4t = dq.tile([P, ns], u8, tag="q4t")
                    nc.sync.dma_start(q4t[:], q4.ap()[kt * P:(kt + 1) * P, :])
                    # stage the group's scale/zero rows at partition 0, then
                    # broadcast across partitions on GPSIMD (APs must start
                    # at partition 0/32/64/96)
                    zrow = dq.tile([1, ns], f32, tag="zrow")
                    nc.sync.dma_start(zrow[:], zro.ap()[kt:kt + 1, :])
                    srow = dq.tile([1, ns], f32, tag="srow")
                    nc.sync.dma_start(srow[:], scl.ap()[kt:kt + 1, :])
                    z_exp = dq.tile([P, ns], f32, tag="z_exp")
                    nc.gpsimd.partition_broadcast(z_exp[:], zrow[:])
                    s_exp = dq.tile([P, ns], f32, tag="s_exp")
                    nc.gpsimd.partition_broadcast(s_exp[:], srow[:])
                    # z_exp <- q4 - z_exp (in place), then w = z_exp * s_exp
                    nc.vector.tensor_tensor(z_exp[:], q4t[:], z_exp[:], SUB)
                    nc.vector.tensor_tensor(w_all[:, kt, :], z_exp[:], s_exp[:], MUL)

            dequant_group(0)
            for blk in range(blk_n):
                outsb = {}
                for ks in range(ks_n):
                    # keep the dequant of the next split ahead of this split's
                    # evictions in every engine's issue order
                    if blk == 0 and ks + 1 < ks_n:
                        dequant_group(ks + 1)
                    for mi in range(mt_block):
                        mt = blk * mt_block + mi
                        xbf = xp.tile([P, ks_group * P], bf16, tag="xbf")
                        nc.sync.dma_start(
                            xbf[:],
                            xt.ap()[(ks * mt_n + mt) * P:(ks * mt_n + mt + 1) * P, :],
                        )
                        if ks == 0:
                            outsb[mi] = op.tile(
                                [P, ns], f32, tag="outsb", name=f"outsb_{blk}_{mi}"
                            )
                        for nstart, sz in chunks:
                            pst = ps.tile([P, MM_FREE], f32, tag="psum")
                            for kl in range(ks_group):
                                nc.tensor.matmul(
                                    pst[:, :sz],
                                    xbf[:, kl * P:(kl + 1) * P],
                                    w_all[:, ks * ks_group + kl, nstart:nstart + sz],
                                    start=(kl == 0),
                                    stop=(kl == ks_group - 1),
                                )
                            osl = outsb[mi][:, nstart:nstart + sz]
                            if ks == 0:
                                nc.any.tensor_tensor(
                                    osl, pst[:, :sz], bias_exp[:, nstart:nstart + sz], ADD
                                )
                            else:
                                nc.any.tensor_tensor(osl, osl, pst[:, :sz], ADD)
                        if ks == ks_n - 1:
                            nc.sync.dma_start(
                                out.ap()[mt * P:(mt + 1) * P, :], outsb[mi][:]
                            )

    nc.compile()
    return nc


def prep_x(x, ks_group=KS_GROUP):
    """bf16 x^T swizzled so each (ks, mt) slab is one contiguous [128, ks_group*128]
    row-block: xt[(ks*mt_n + mt)*128 + p, kl*128 + j] = x[mt*128 + j, (ks*ks_group + kl)*128 + p]
    """
    m, k = x.shape
    kt_n, mt_n = k // P, m // P
    ks_n = kt_n // ks_group
    xbf = x.astype(ml_dtypes.bfloat16)
    # [mt, j, ks, kl, p] -> [ks, mt, p, kl, j]
    xs = xbf.reshape(mt_n, P, ks_n, ks_group, P).transpose(2, 0, 4, 3, 1)
    return np.ascontiguousarray(xs.reshape(ks_n * mt_n * P, ks_group * P))


def prep_inputs(x, qweight, qscales, qzeros, bias):
    """Host-side shard/layout prep. Returns per-core input maps."""
    x = np.asarray(x)
    qweight = np.asarray(qweight)
    qscales = np.asarray(qscales)
    qzeros = np.asarray(qzeros)
    bias = np.asarray(bias)

    xprep = prep_x(x)

    # Unpack int4 nibbles into k-major uint8 [K, N]:
    # even k -> low nibble, odd k -> high nibble of byte qweight[n, k//2]
    b = qweight.astype(np.uint8)              # [N, K//2]
    q4 = np.empty((K, NFULL), np.uint8)
    q4[0::2, :] = (b & 15).T
    q4[1::2, :] = (b >> 4).T

    sT = np.ascontiguousarray(qscales.astype(np.float32).T)   # [G, N]
    zT = np.ascontiguousarray(qzeros.astype(np.float32).T)    # [G, N]
    bias2d = bias.astype(np.float32).reshape(1, NFULL)

    in_maps = []
    for c in range(NCORES):
        sl = slice(c * NS, (c + 1) * NS)
        in_maps.append(
            {
                "xt": xprep,
                "q4": np.ascontiguousarray(q4[:, sl]),
                "scl": np.ascontiguousarray(sT[:, sl]),
                "zro": np.ascontiguousarray(zT[:, sl]),
                "bias": np.ascontiguousarray(bias2d[:, sl]),
            }
        )
    return in_maps


def kernel(x, qweight, qscales, qzeros, bias):
    global LAST_RESULTS
    from concourse.bass_utils import run_bass_kernel_spmd

    nc = build_nc()
    in_maps = prep_inputs(x, qweight, qscales, qzeros, bias)
    trace = bool(os.environ.get("BASS_AWQ_TRACE"))
    res = run_bass_kernel_spmd(
        nc,
        in_maps,
        core_ids=list(range(NCORES)),
        trace=trace,
        trace_cores=list(range(NCORES)) if trace else None,
    )
    LAST_RESULTS = res
    return np.concatenate([res.results[c]["out"] for c in range(NCORES)], axis=1)


# revision 8
# speedup vs baseline: 1.3791x; 1.3791x over previous
"""AWQ linear (int4 group-quantized) matmul on 8 Trainium2 NeuronCores.

out[m, n] = sum_k x[m, k] * W[n, k] + bias[n]
W[n, k] = (q4[n, k] - qzeros[n, k//128]) * qscales[n, k//128]

Column-parallel: shard N=11008 across 8 cores (1376 each), replicate x.
Per core:
  - host repacks qweight nibbles to a k-major bf16 tensor [K, Ns] (small
    ints, exact in bf16) and swizzles x^T (bf16) so every (k-group,
    m-tile) slab is one contiguous DMA
  - device dequantizes W^T[k, n] = (q4 - z)*s into resident SBUF bf16:
    scale/zero rows are DMA-broadcast across partitions (bf16), DVE does
    the two-tensor affine entirely in bf16 (fast path); dequant DMAs are
    issued from the otherwise-idle ACT sequencer so they can never stall
    the Sync queue that feeds x/out traffic
  - matmuls accumulate over k in asymmetric splits (8/8/16 k-tiles) so
    the PE starts ~35us in while later groups still dequantize; partial
    sums accumulate in SBUF via DVE adds, bias fused into split 0;
    dequant DVE work for the next split is interleaved between evictions
"""

import os

import numpy as np
import ml_dtypes

M, K, NFULL = 4096, 4096, 11008
NCORES = 8
NS = NFULL // NCORES          # 1376 out-features per core
P = 128                       # partitions; also the quant group size
MM_FREE = 512                 # psum bank limit (fp32)
XG = 8                        # k-tiles per x-slab group in the host layout

LAST_RESULTS = None           # BassKernelResults of the last kernel() call


def build_nc(k=K, m=M, ns=NS, n_cores=NCORES, splits=(8, 8, 16), mt_block=8, xg=XG):
    """Build + compile the per-core Bass program (SPMD: same NEFF on all cores)."""
    import concourse.mybir as mybir
    import concourse.tile as tile
    from concourse import bacc

    kt_n = k // P
    mt_n = m // P
    assert sum(splits) == kt_n and mt_n % mt_block == 0
    assert all(s % xg == 0 for s in splits)
    s_n = len(splits)
    s_start = [sum(splits[:i]) for i in range(s_n)]
    blk_n = mt_n // mt_block
    chunks = [(i, min(MM_FREE, ns - i)) for i in range(0, ns, MM_FREE)]

    f32 = mybir.dt.float32
    bf16 = mybir.dt.bfloat16
    ADD = mybir.AluOpType.add
    SUB = mybir.AluOpType.subtract
    MUL = mybir.AluOpType.mult

    nc = bacc.Bacc("TRN2", num_devices=n_cores)
    # xt rows are (kg, mt, p): each (kg, mt) slab is contiguous [128, xg*128]
    xt = nc.dram_tensor("xt", [(kt_n // xg) * mt_n * P, xg * P], bf16, kind="ExternalInput")
    q4 = nc.dram_tensor("q4", [k, ns], bf16, kind="ExternalInput")
    scl = nc.dram_tensor("scl", [kt_n, ns], bf16, kind="ExternalInput")
    zro = nc.dram_tensor("zro", [kt_n, ns], bf16, kind="ExternalInput")
    bias = nc.dram_tensor("bias", [1, ns], f32, kind="ExternalInput")
    out = nc.dram_tensor("out", [m, ns], f32, kind="ExternalOutput")

    with tile.TileContext(nc) as tc:
        with (
            tc.tile_pool(name="persist", bufs=1) as persist,
            tc.tile_pool(name="dq", bufs=3) as dq,
            tc.tile_pool(name="xp", bufs=2) as xp,
            tc.tile_pool(name="op", bufs=mt_block + 1) as op,
            tc.tile_pool(name="ps", bufs=6, space="PSUM") as ps,
        ):
            w_all = persist.tile([P, kt_n, ns], bf16)
            bias_exp = persist.tile([P, ns], f32)
            nc.sync.dma_start(bias_exp[:], bias.ap().to_broadcast((P, ns)))

            def dequant_kt(kt):
                # dequant DMAs ride the ACT queue: slot backpressure here can
                # never stall the Sync queue carrying x/out traffic
                q4t = dq.tile([P, ns], bf16, tag="q4t")
                nc.scalar.dma_start(q4t[:], q4.ap()[kt * P:(kt + 1) * P, :])
                z_exp = dq.tile([P, ns], bf16, tag="z_exp")
                nc.scalar.dma_start(
                    z_exp[:], zro.ap()[kt:kt + 1, :].to_broadcast((P, ns))
                )
                s_exp = dq.tile([P, ns], bf16, tag="s_exp")
                nc.scalar.dma_start(
                    s_exp[:], scl.ap()[kt:kt + 1, :].to_broadcast((P, ns))
                )
                tmp = dq.tile([P, ns], bf16, tag="dqtmp")
                nc.vector.tensor_tensor(tmp[:], q4t[:], z_exp[:], SUB)
                nc.vector.tensor_tensor(w_all[:, kt, :], tmp[:], s_exp[:], MUL)

            def x_slab(si, mt):
                """Load the x slab for split si, m-tile mt: [128, ng, xg*128]."""
                ng = splits[si] // xg
                g0 = s_start[si] // xg
                xbf = xp.tile([P, ng, xg * P], bf16, tag=f"xbf{si}")
                for gi in range(ng):
                    r0 = ((g0 + gi) * mt_n + mt) * P
                    nc.sync.dma_start(xbf[:, gi, :], xt.ap()[r0:r0 + P, :])
                return xbf

            for kt in range(s_start[1]):
                dequant_kt(kt)

            for blk in range(blk_n):
                outsb = {}
                for si in range(s_n):
                    # dequant DVE work for split si+1 is spread between this
                    # split's evictions (block 0 only; all W ready afterwards)
                    pending = (
                        list(range(s_start[si + 1], s_start[si + 1] + splits[si + 1]))
                        if blk == 0 and si + 1 < s_n
                        else []
                    )
                    per_mi = (len(pending) + mt_block - 1) // mt_block if pending else 0
                    for mi in range(mt_block):
                        mt = blk * mt_block + mi
                        xbf = x_slab(si, mt)
                        if si == 0:
                            outsb[mi] = op.tile(
                                [P, ns], f32, tag="outsb", name=f"outsb_{blk}_{mi}"
                            )
                        for nstart, sz in chunks:
                            pst = ps.tile([P, MM_FREE], f32, tag="psum")
                            for kl in range(splits[si]):
                                nc.tensor.matmul(
                                    pst[:, :sz],
                                    xbf[:, kl // xg, (kl % xg) * P:(kl % xg + 1) * P],
                                    w_all[:, s_start[si] + kl, nstart:nstart + sz],
                                    start=(kl == 0),
                                    stop=(kl == splits[si] - 1),
                                )
                            osl = outsb[mi][:, nstart:nstart + sz]
                            if si == 0:
                                nc.vector.tensor_tensor(
                                    osl, pst[:, :sz], bias_exp[:, nstart:nstart + sz], ADD
                                )
                            else:
                                nc.vector.tensor_tensor(osl, osl, pst[:, :sz], ADD)
                        for kt in pending[mi * per_mi:(mi + 1) * per_mi]:
                            dequant_kt(kt)
                        if si == s_n - 1:
                            nc.sync.dma_start(
                                out.ap()[mt * P:(mt + 1) * P, :], outsb[mi][:]
                            )

    nc.compile()
    return nc


def prep_x(x, xg=XG):
    """bf16 x^T swizzled so each (kg, mt) slab is one contiguous [128, xg*128]
    row-block: xt[(kg*mt_n + mt)*128 + p, kl*128 + j] = x[mt*128 + j, (kg*xg + kl)*128 + p]
    """
    m, k = x.shape
    kt_n, mt_n = k // P, m // P
    kg_n = kt_n // xg
    xbf = x.astype(ml_dtypes.bfloat16)
    # [mt, j, kg, kl, p] -> [kg, mt, p, kl, j]
    xs = xbf.reshape(mt_n, P, kg_n, xg, P).transpose(2, 0, 4, 3, 1)
    return np.ascontiguousarray(xs.reshape(kg_n * mt_n * P, xg * P))


def prep_inputs(x, qweight, qscales, qzeros, bias):
    """Host-side shard/layout prep. Returns per-core input maps."""
    x = np.asarray(x)
    qweight = np.asarray(qweight)
    qscales = np.asarray(qscales)
    qzeros = np.asarray(qzeros)
    bias = np.asarray(bias)

    xprep = prep_x(x)

    # Unpack int4 nibbles into k-major bf16 [K, N] (ints 0..15: exact):
    # even k -> low nibble, odd k -> high nibble of byte qweight[n, k//2]
    b = qweight.astype(np.uint8)              # [N, K//2]
    q4 = np.empty((K, NFULL), ml_dtypes.bfloat16)
    q4[0::2, :] = (b & 15).T
    q4[1::2, :] = (b >> 4).T

    sT = np.ascontiguousarray(qscales.astype(ml_dtypes.bfloat16).T)   # [G, N]
    zT = np.ascontiguousarray(qzeros.astype(ml_dtypes.bfloat16).T)    # [G, N]
    bias2d = bias.astype(np.float32).reshape(1, NFULL)

    in_maps = []
    for c in range(NCORES):
        sl = slice(c * NS, (c + 1) * NS)
        in_maps.append(
            {
                "xt": xprep,
                "q4": np.ascontiguousarray(q4[:, sl]),
                "scl": np.ascontiguousarray(sT[:, sl]),
                "zro": np.ascontiguousarray(zT[:, sl]),
                "bias": np.ascontiguousarray(bias2d[:, sl]),
            }
        )
    return in_maps


def kernel(x, qweight, qscales, qzeros, bias):
    global LAST_RESULTS
    from concourse.bass_utils import run_bass_kernel_spmd

    nc = build_nc()
    in_maps = prep_inputs(x, qweight, qscales, qzeros, bias)
    trace = bool(os.environ.get("BASS_AWQ_TRACE"))
    res = run_bass_kernel_spmd(
        nc,
        in_maps,
        core_ids=list(range(NCORES)),
        trace=trace,
        trace_cores=list(range(NCORES)) if trace else None,
    )
    LAST_RESULTS = res
    return np.concatenate([res.results[c]["out"] for c in range(NCORES)], axis=1)


# revision 9
# speedup vs baseline: 1.3837x; 1.0033x over previous
"""AWQ linear (int4 group-quantized) matmul on 8 Trainium2 NeuronCores.

out[m, n] = sum_k x[m, k] * W[n, k] + bias[n]
W[n, k] = (q4[n, k] - qzeros[n, k//128]) * qscales[n, k//128]

Column-parallel: shard N=11008 across 8 cores (1376 each), replicate x.
Per core:
  - host repacks qweight nibbles to a k-major bf16 tensor [K, Ns] (small
    ints, exact in bf16) and swizzles x^T (bf16) so every (k-group,
    m-tile) slab is one contiguous DMA
  - device dequantizes W^T[k, n] = (q4 - z)*s into resident SBUF bf16:
    scale/zero rows are DMA-broadcast across partitions (bf16), DVE does
    the two-tensor affine entirely in bf16 (fast path); dequant DMAs are
    issued from the otherwise-idle ACT sequencer so they can never stall
    the Sync queue that feeds x/out traffic
  - matmuls accumulate over k in asymmetric splits (8/8/16 k-tiles) so
    the PE starts ~35us in while later groups still dequantize; partial
    sums accumulate in SBUF via DVE adds, bias fused into split 0;
    dequant DVE work for the next split is interleaved between evictions
"""

import os

import numpy as np
import ml_dtypes

M, K, NFULL = 4096, 4096, 11008
NCORES = 8
NS = NFULL // NCORES          # 1376 out-features per core
P = 128                       # partitions; also the quant group size
MM_FREE = 512                 # psum bank limit (fp32)
XG = 8                        # k-tiles per x-slab group in the host layout

LAST_RESULTS = None           # BassKernelResults of the last kernel() call


def build_nc(k=K, m=M, ns=NS, n_cores=NCORES, splits=(8, 8, 16), mt_block=8, xg=XG):
    """Build + compile the per-core Bass program (SPMD: same NEFF on all cores)."""
    import concourse.mybir as mybir
    import concourse.tile as tile
    from concourse import bacc

    kt_n = k // P
    mt_n = m // P
    assert sum(splits) == kt_n and mt_n % mt_block == 0
    assert all(s % xg == 0 for s in splits)
    s_n = len(splits)
    s_start = [sum(splits[:i]) for i in range(s_n)]
    blk_n = mt_n // mt_block
    chunks = [(i, min(MM_FREE, ns - i)) for i in range(0, ns, MM_FREE)]

    f32 = mybir.dt.float32
    bf16 = mybir.dt.bfloat16
    ADD = mybir.AluOpType.add
    SUB = mybir.AluOpType.subtract
    MUL = mybir.AluOpType.mult

    nc = bacc.Bacc("TRN2", num_devices=n_cores)
    # xt rows are (kg, mt, p): each (kg, mt) slab is contiguous [128, xg*128]
    xt = nc.dram_tensor("xt", [(kt_n // xg) * mt_n * P, xg * P], bf16, kind="ExternalInput")
    q4 = nc.dram_tensor("q4", [k, ns], bf16, kind="ExternalInput")
    scl = nc.dram_tensor("scl", [kt_n, ns], bf16, kind="ExternalInput")
    zro = nc.dram_tensor("zro", [kt_n, ns], bf16, kind="ExternalInput")
    bias = nc.dram_tensor("bias", [1, ns], f32, kind="ExternalInput")
    out = nc.dram_tensor("out", [m, ns], f32, kind="ExternalOutput")

    with tile.TileContext(nc) as tc:
        with (
            tc.tile_pool(name="persist", bufs=1) as persist,
            tc.tile_pool(name="dq", bufs=3) as dq,
            tc.tile_pool(name="xp", bufs=2) as xp,
            tc.tile_pool(name="op", bufs=mt_block + 1) as op,
            tc.tile_pool(name="ps", bufs=6, space="PSUM") as ps,
        ):
            w_all = persist.tile([P, kt_n, ns], bf16)
            bias_exp = persist.tile([P, ns], f32)
            nc.sync.dma_start(bias_exp[:], bias.ap().to_broadcast((P, ns)))

            def dequant_kt(kt):
                # spread dequant traffic over three independent DMA paths so
                # delivery outpaces the PE's split consumption: q4 on the
                # SWDGE rings (gpsimd), zeros on the Sync HWDGE queue
                # (interleaved with x slabs), scales on the ACT HWDGE queue
                q4t = dq.tile([P, ns], bf16, tag="q4t")
                nc.gpsimd.dma_start(q4t[:], q4.ap()[kt * P:(kt + 1) * P, :])
                z_exp = dq.tile([P, ns], bf16, tag="z_exp")
                nc.sync.dma_start(
                    z_exp[:], zro.ap()[kt:kt + 1, :].to_broadcast((P, ns))
                )
                s_exp = dq.tile([P, ns], bf16, tag="s_exp")
                nc.scalar.dma_start(
                    s_exp[:], scl.ap()[kt:kt + 1, :].to_broadcast((P, ns))
                )
                tmp = dq.tile([P, ns], bf16, tag="dqtmp")
                nc.vector.tensor_tensor(tmp[:], q4t[:], z_exp[:], SUB)
                nc.vector.tensor_tensor(w_all[:, kt, :], tmp[:], s_exp[:], MUL)

            def x_slab(si, mt):
                """Load the x slab for split si, m-tile mt: [128, ng, xg*128]."""
                ng = splits[si] // xg
                g0 = s_start[si] // xg
                xbf = xp.tile([P, ng, xg * P], bf16, tag=f"xbf{si}")
                for gi in range(ng):
                    r0 = ((g0 + gi) * mt_n + mt) * P
                    nc.sync.dma_start(xbf[:, gi, :], xt.ap()[r0:r0 + P, :])
                return xbf

            for kt in range(s_start[1]):
                dequant_kt(kt)

            for blk in range(blk_n):
                outsb = {}
                for si in range(s_n):
                    # dequant DVE work for split si+1 is spread between this
                    # split's evictions (block 0 only; all W ready afterwards)
                    pending = (
                        list(range(s_start[si + 1], s_start[si + 1] + splits[si + 1]))
                        if blk == 0 and si + 1 < s_n
                        else []
                    )
                    per_mi = (len(pending) + mt_block - 1) // mt_block if pending else 0
                    for mi in range(mt_block):
                        mt = blk * mt_block + mi
                        xbf = x_slab(si, mt)
                        if si == 0:
                            outsb[mi] = op.tile(
                                [P, ns], f32, tag="outsb", name=f"outsb_{blk}_{mi}"
                            )
                        for nstart, sz in chunks:
                            pst = ps.tile([P, MM_FREE], f32, tag="psum")
                            for kl in range(splits[si]):
                                nc.tensor.matmul(
                                    pst[:, :sz],
                                    xbf[:, kl // xg, (kl % xg) * P:(kl % xg + 1) * P],
                                    w_all[:, s_start[si] + kl, nstart:nstart + sz],
                                    start=(kl == 0),
                                    stop=(kl == splits[si] - 1),
                                )
                            osl = outsb[mi][:, nstart:nstart + sz]
                            if si == 0:
                                nc.vector.tensor_tensor(
                                    osl, pst[:, :sz], bias_exp[:, nstart:nstart + sz], ADD
                                )
                            else:
                                nc.vector.tensor_tensor(osl, osl, pst[:, :sz], ADD)
                        for kt in pending[mi * per_mi:(mi + 1) * per_mi]:
                            dequant_kt(kt)
                        if si == s_n - 1:
                            nc.sync.dma_start(
                                out.ap()[mt * P:(mt + 1) * P, :], outsb[mi][:]
                            )

    nc.compile()
    return nc


def prep_x(x, xg=XG):
    """bf16 x^T swizzled so each (kg, mt) slab is one contiguous [128, xg*128]
    row-block: xt[(kg*mt_n + mt)*128 + p, kl*128 + j] = x[mt*128 + j, (kg*xg + kl)*128 + p]
    """
    m, k = x.shape
    kt_n, mt_n = k // P, m // P
    kg_n = kt_n // xg
    xbf = x.astype(ml_dtypes.bfloat16)
    # [mt, j, kg, kl, p] -> [kg, mt, p, kl, j]
    xs = xbf.reshape(mt_n, P, kg_n, xg, P).transpose(2, 0, 4, 3, 1)
    return np.ascontiguousarray(xs.reshape(kg_n * mt_n * P, xg * P))


def prep_inputs(x, qweight, qscales, qzeros, bias):
    """Host-side shard/layout prep. Returns per-core input maps."""
    x = np.asarray(x)
    qweight = np.asarray(qweight)
    qscales = np.asarray(qscales)
    qzeros = np.asarray(qzeros)
    bias = np.asarray(bias)

    xprep = prep_x(x)

    # Unpack int4 nibbles into k-major bf16 [K, N] (ints 0..15: exact):
    # even k -> low nibble, odd k -> high nibble of byte qweight[n, k//2]
    b = qweight.astype(np.uint8)              # [N, K//2]
    q4 = np.empty((K, NFULL), ml_dtypes.bfloat16)
    q4[0::2, :] = (b & 15).T
    q4[1::2, :] = (b >> 4).T

    sT = np.ascontiguousarray(qscales.astype(ml_dtypes.bfloat16).T)   # [G, N]
    zT = np.ascontiguousarray(qzeros.astype(ml_dtypes.bfloat16).T)    # [G, N]
    bias2d = bias.astype(np.float32).reshape(1, NFULL)

    in_maps = []
    for c in range(NCORES):
        sl = slice(c * NS, (c + 1) * NS)
        in_maps.append(
            {
                "xt": xprep,
                "q4": np.ascontiguousarray(q4[:, sl]),
                "scl": np.ascontiguousarray(sT[:, sl]),
                "zro": np.ascontiguousarray(zT[:, sl]),
                "bias": np.ascontiguousarray(bias2d[:, sl]),
            }
        )
    return in_maps


def kernel(x, qweight, qscales, qzeros, bias):
    global LAST_RESULTS
    from concourse.bass_utils import run_bass_kernel_spmd

    nc = build_nc()
    in_maps = prep_inputs(x, qweight, qscales, qzeros, bias)
    trace = bool(os.environ.get("BASS_AWQ_TRACE"))
    res = run_bass_kernel_spmd(
        nc,
        in_maps,
        core_ids=list(range(NCORES)),
        trace=trace,
        trace_cores=list(range(NCORES)) if trace else None,
    )
    LAST_RESULTS = res
    return np.concatenate([res.results[c]["out"] for c in range(NCORES)], axis=1)


# revision 21
# speedup vs baseline: 1.3877x; 1.0029x over previous
"""AWQ linear (int4 group-quantized) matmul on 8 Trainium2 NeuronCores.

out[m, n] = sum_k x[m, k] * W[n, k] + bias[n]
W[n, k] = (q4[n, k] - qzeros[n, k//128]) * qscales[n, k//128]

Column-parallel: shard N=11008 across 8 cores (1376 each), replicate x.
Per core:
  - host repacks qweight nibbles to a k-major bf16 tensor [K, Ns] (small
    ints, exact in bf16) and swizzles x^T (bf16) so every (k-group,
    m-tile) slab is one contiguous DMA
  - device dequantizes W^T[k, n] = (q4 - z)*s into resident SBUF bf16:
    scale/zero rows are DMA-broadcast across partitions (bf16), DVE does
    the two-tensor affine entirely in bf16 (fast path); dequant DMAs are
    issued from the otherwise-idle ACT sequencer so they can never stall
    the Sync queue that feeds x/out traffic
  - matmuls accumulate over k in asymmetric splits (8/8/16 k-tiles) so
    the PE starts ~35us in while later groups still dequantize; partial
    sums accumulate in SBUF via DVE adds, bias fused into split 0;
    dequant DVE work for the next split is interleaved between evictions
"""

import os

import numpy as np
import ml_dtypes

M, K, NFULL = 4096, 4096, 11008
NCORES = 8
NS = NFULL // NCORES          # 1376 out-features per core
P = 128                       # partitions; also the quant group size
MM_FREE = 512                 # psum bank limit (fp32)
XG = 8                        # k-tiles per x-slab group in the host layout

LAST_RESULTS = None           # BassKernelResults of the last kernel() call


def build_nc(k=K, m=M, ns=NS, n_cores=NCORES, splits=(8, 8, 16), mt_block=6, xg=XG):
    """Build + compile the per-core Bass program (SPMD: same NEFF on all cores).

    Block 0 accumulates over k in `splits` (so the PE can start while later
    k-groups still dequantize); the remaining blocks run one full-k span.
    """
    import concourse.bass as bass
    import concourse.mybir as mybir
    import concourse.tile as tile
    from concourse import bacc

    kt_n = k // P
    mt_n = m // P
    assert sum(splits) == kt_n and mt_block <= mt_n
    assert all(s % xg == 0 for s in splits) and all(s % 2 == 0 for s in splits)
    chunks = [(i, min(MM_FREE, ns - i)) for i in range(0, ns, MM_FREE)]

    f32 = mybir.dt.float32
    bf16 = mybir.dt.bfloat16
    ADD = mybir.AluOpType.add
    SUB = mybir.AluOpType.subtract
    MUL = mybir.AluOpType.mult

    nc = bacc.Bacc("TRN2", num_devices=n_cores)
    # xt rows are (kg, mt, p): each (kg, mt) slab is contiguous [128, xg*128]
    xt = nc.dram_tensor("xt", [(kt_n // xg) * mt_n * P, xg * P], bf16, kind="ExternalInput")
    # q4 rows are partitions: q4[p, kt*ns + n] = q4_kmajor[kt*128 + p, n]
    q4 = nc.dram_tensor("q4", [P, kt_n * ns], bf16, kind="ExternalInput")
    # scale/zero rows interleaved: row 2g = scales[g], row 2g+1 = zeros[g]
    szt = nc.dram_tensor("sz", [2 * kt_n, ns], bf16, kind="ExternalInput")
    bias = nc.dram_tensor("bias", [1, ns], f32, kind="ExternalInput")
    out = nc.dram_tensor("out", [m, ns], f32, kind="ExternalOutput")

    with tile.TileContext(nc) as tc:
        with (
            tc.tile_pool(name="persist", bufs=1) as persist,
            tc.tile_pool(name="dq", bufs=2) as dq,
            tc.tile_pool(name="dqt", bufs=1) as dqt,
            tc.tile_pool(name="xp1", bufs=2) as xp1,
            tc.tile_pool(name="xp2", bufs=4) as xp2,
            tc.tile_pool(name="op", bufs=mt_block + 1) as op,
            tc.tile_pool(name="ps", bufs=6, space="PSUM") as ps,
        ):
            w_all = persist.tile([P, kt_n, ns], bf16)
            bias_exp = persist.tile([P, ns], f32)

            def dequant_pair(i):
                """Dequantize k-tiles 2i and 2i+1. DMA descriptor batching:
                one SWDGE load covers both q4 tiles (contiguous per
                partition), one ACT broadcast covers all four scale/zero
                rows -- broadcasts are descriptor-rate-limited, so bytes
                per descriptor is what matters."""
                kt = 2 * i
                q4sl = dq.tile([P, 2, ns], bf16, tag="q4sl")
                nc.gpsimd.dma_start(
                    q4sl[:], q4.ap()[:, kt * ns:(kt + 2) * ns].rearrange(
                        "p (j n) -> p j n", j=2
                    )
                )
                szx = dq.tile([P, 4, ns], bf16, tag="szx")
                src = szt.ap()[2 * kt:2 * kt + 4, :]
                src = bass.AP(src.tensor, src.offset, [[0, P]] + list(src.ap))
                nc.scalar.dma_start(szx[:], src)
                for j in range(2):
                    tmp = dqt.tile([P, ns], bf16, tag="dqtmp")
                    nc.vector.tensor_tensor(
                        tmp[:], q4sl[:, j, :], szx[:, 2 * j + 1, :], SUB
                    )
                    nc.vector.tensor_tensor(
                        w_all[:, kt + j, :], tmp[:], szx[:, 2 * j, :], MUL
                    )

            def x_slab(g0, ng, mt):
                """Load x k-groups g0..g0+ng-1 for m-tile mt: [128, ng, xg*128]."""
                pool = xp1 if ng == 1 else xp2
                xbf = pool.tile([P, ng, xg * P], bf16, tag=f"xbf{ng}")
                for gi in range(ng):
                    r0 = ((g0 + gi) * mt_n + mt) * P
                    nc.sync.dma_start(xbf[:, gi, :], xt.ap()[r0:r0 + P, :])
                return xbf

            def mm_sweep(pst, sz_args, kt0, n_kt, slabs, slab_kts):
                """Accumulate kt0..kt0+n_kt-1 into pst from the given x slabs."""
                nstart, sz = sz_args
                for kl in range(n_kt):
                    kt = kt0 + kl
                    sb_i = next(i for i, (a, b) in enumerate(slab_kts) if a <= kt < b)
                    loc = kt - slab_kts[sb_i][0]
                    nc.tensor.matmul(
                        pst[:, :sz],
                        slabs[sb_i][:, loc // xg, (loc % xg) * P:(loc % xg + 1) * P],
                        w_all[:, kt, nstart:nstart + sz],
                        start=(kl == 0),
                        stop=(kl == n_kt - 1),
                    )

            for i in range(splits[0] // 2):
                dequant_pair(i)
            nc.scalar.dma_start(bias_exp[:], bias.ap().to_broadcast((P, ns)))

            s_n = len(splits)
            s_start = [sum(splits[:i]) for i in range(s_n)]

            # ---- block 0: k-split sweeps, dequant interleaved ----
            outsb = {}
            for si in range(s_n):
                pending = (
                    list(range(s_start[si + 1] // 2,
                               (s_start[si + 1] + splits[si + 1]) // 2))
                    if si + 1 < s_n
                    else []
                )
                per_mi = (len(pending) + mt_block - 1) // mt_block if pending else 0
                for mi in range(mt_block):
                    mt = mi
                    xbf = x_slab(s_start[si] // xg, splits[si] // xg, mt)
                    span = (s_start[si], s_start[si] + splits[si])
                    if si == 0:
                        outsb[mi] = op.tile(
                            [P, ns], f32, tag="outsb", name=f"outsb_0_{mi}"
                        )
                    for nstart, sz in chunks:
                        pst = ps.tile([P, MM_FREE], f32, tag="psum")
                        mm_sweep(pst, (nstart, sz), span[0], splits[si], [xbf], [span])
                        osl = outsb[mi][:, nstart:nstart + sz]
                        if si == 0:
                            nc.vector.tensor_tensor(
                                osl, pst[:, :sz], bias_exp[:, nstart:nstart + sz], ADD
                            )
                        else:
                            nc.vector.tensor_tensor(osl, osl, pst[:, :sz], ADD)
                    for i in pending[mi * per_mi:(mi + 1) * per_mi]:
                        dequant_pair(i)
                    if si == s_n - 1:
                        nc.sync.dma_start(
                            out.ap()[mt * P:(mt + 1) * P, :], outsb[mi][:]
                        )

            # ---- blocks 1+: full-k accumulation spans ----
            half = kt_n // 2
            for mt in range(mt_block, mt_n):
                slabs = [x_slab(0, half // xg, mt), x_slab(half // xg, half // xg, mt)]
                slab_kts = [(0, half), (half, kt_n)]
                osb = op.tile([P, ns], f32, tag="outsb", name=f"outsb_{mt}")
                for nstart, sz in chunks:
                    pst = ps.tile([P, MM_FREE], f32, tag="psum")
                    mm_sweep(pst, (nstart, sz), 0, kt_n, slabs, slab_kts)
                    nc.vector.tensor_tensor(
                        osb[:, nstart:nstart + sz],
                        pst[:, :sz],
                        bias_exp[:, nstart:nstart + sz],
                        ADD,
                    )
                nc.sync.dma_start(out.ap()[mt * P:(mt + 1) * P, :], osb[:])

    nc.compile()
    return nc


def prep_x(x, xg=XG):
    """bf16 x^T swizzled so each (kg, mt) slab is one contiguous [128, xg*128]
    row-block: xt[(kg*mt_n + mt)*128 + p, kl*128 + j] = x[mt*128 + j, (kg*xg + kl)*128 + p]
    """
    m, k = x.shape
    kt_n, mt_n = k // P, m // P
    kg_n = kt_n // xg
    xbf = x.astype(ml_dtypes.bfloat16)
    # [mt, j, kg, kl, p] -> [kg, mt, p, kl, j]
    xs = xbf.reshape(mt_n, P, kg_n, xg, P).transpose(2, 0, 4, 3, 1)
    return np.ascontiguousarray(xs.reshape(kg_n * mt_n * P, xg * P))


def prep_inputs(x, qweight, qscales, qzeros, bias):
    """Host-side shard/layout prep. Returns per-core input maps."""
    x = np.asarray(x)
    qweight = np.asarray(qweight)
    qscales = np.asarray(qscales)
    qzeros = np.asarray(qzeros)
    bias = np.asarray(bias)

    xprep = prep_x(x)

    # Unpack int4 nibbles into k-major bf16 [K, N] (ints 0..15: exact):
    # even k -> low nibble, odd k -> high nibble of byte qweight[n, k//2]
    b = qweight.astype(np.uint8)              # [N, K//2]
    q4 = np.empty((K, NFULL), ml_dtypes.bfloat16)
    q4[0::2, :] = (b & 15).T
    q4[1::2, :] = (b >> 4).T
    kt_n = K // P
    # partition-major: q4p[p, kt, n] = q4[kt*128 + p, n]
    q4p = np.ascontiguousarray(q4.reshape(kt_n, P, NFULL).transpose(1, 0, 2))

    sT = qscales.astype(ml_dtypes.bfloat16).T   # [G, N]
    zT = qzeros.astype(ml_dtypes.bfloat16).T    # [G, N]
    sz = np.empty((2 * kt_n, NFULL), ml_dtypes.bfloat16)
    sz[0::2, :] = sT
    sz[1::2, :] = zT
    bias2d = bias.astype(np.float32).reshape(1, NFULL)

    in_maps = []
    for c in range(NCORES):
        sl = slice(c * NS, (c + 1) * NS)
        in_maps.append(
            {
                "xt": xprep,
                "q4": np.ascontiguousarray(q4p[:, :, sl]).reshape(P, kt_n * NS),
                "sz": np.ascontiguousarray(sz[:, sl]),
                "bias": np.ascontiguousarray(bias2d[:, sl]),
            }
        )
    return in_maps


def kernel(x, qweight, qscales, qzeros, bias):
    global LAST_RESULTS
    from concourse.bass_utils import run_bass_kernel_spmd

    nc = build_nc()
    in_maps = prep_inputs(x, qweight, qscales, qzeros, bias)
    trace = bool(os.environ.get("BASS_AWQ_TRACE"))
    res = run_bass_kernel_spmd(
        nc,
        in_maps,
        core_ids=list(range(NCORES)),
        trace=trace,
        trace_cores=list(range(NCORES)) if trace else None,
    )
    LAST_RESULTS = res
    return np.concatenate([res.results[c]["out"] for c in range(NCORES)], axis=1)
